# revision 1
# baseline (speedup 1.0000x reference)
"""BlockSparseMLP (MoE top-2 routing, 8 experts) — Trainium2 Bass kernel.

Strategy (expert-parallel, per sharding hint): one expert per NeuronCore.
Host-side (numpy): router (x @ gate_tensor, softmax, top-2, renormalize),
token dispatch (gather the tokens routed to each expert, transposed to
feature-major and pre-swizzled into the SBUF block layout), and the final
combine (scatter-add of the per-expert partial outputs) — the
shard/unshard stage.

Device-side (one SPMD Bass/Tile program on 8 cores): the expert gated MLP
   gT = Wg_e.T @ xT_e   (bf16 matmuls, fp32 PSUM accumulate)
   uT = Wu_e.T @ xT_e
   aT = silu(gT) * uT   (bf16 in SBUF)
   dT = Wd_e.T @ aT
   outT = dT * w_e      (combine weights folded in on-chip)

All tensors are cast fp32 -> bf16 inside the DMA datapath (SWDGE cast):
HBM traffic stays the input's fp32 bytes but SBUF/PE run bf16.  Weights
are pre-swizzled on the host into per-DMA-block partition-major layout so
every SWDGE transfer reads large contiguous chunks (full descriptor
efficiency); the token axis is the matmul moving dimension, split into
chunks <= 512 (PSUM bank limit).
"""

import os

import numpy as np

T, D, F, E, TOPK = 2048, 2048, 5632, 8, 2
P = 128
KD = D // P     # 16 k-subtiles over D
KF = F // P     # 44 k-subtiles over F
FG = 4          # f-tiles per phase-1 weight DMA block (512 F columns)
NFG = KF // FG  # 11 phase-1 blocks
DG = 2          # d-tiles per phase-2 psum group (256 D columns)
NDG = KD // DG  # 8 phase-2 d-groups
KO2 = 4         # f-subtiles per phase-2 weight DMA block
NFB = KF // KO2  # 11 phase-2 blocks per d-group

_COMPILED = {}   # CAP -> (nc, chunk list)
LAST_RESULT = None  # BassKernelResults of the most recent run (for test.py)


def _token_chunks(cap):
    """Split cap into free-dim chunks, each in [256, 512]."""
    assert cap >= 512 and cap % 2 == 0
    n512, rem = divmod(cap, 512)
    if rem == 0:
        return [512] * n512
    if rem >= 256:
        return [512] * n512 + [rem]
    return [512] * (n512 - 1) + [256 + rem, 256]


def _build(cap):
    """Build + compile the SPMD Tile program for token capacity `cap`."""
    import concourse.bass as bass  # noqa: F401
    import concourse.mybir as mybir
    import concourse.tile as tile
    from concourse import bacc

    f32 = mybir.dt.float32
    bf16 = mybir.dt.bfloat16
    mult = mybir.AluOpType.mult

    chunks = _token_chunks(cap)
    starts = [sum(chunks[:i]) for i in range(len(chunks))]

    nc = bacc.Bacc("TRN2", target_bir_lowering=False, debug=False,
                   enable_asserts=False, num_devices=E)

    xt_d = nc.dram_tensor("xt", [P, KD, cap], f32, kind="ExternalInput").ap()
    wg_d = nc.dram_tensor("wg", [NFG, P, KD, P * FG], f32,
                          kind="ExternalInput").ap()
    wu_d = nc.dram_tensor("wu", [NFG, P, KD, P * FG], f32,
                          kind="ExternalInput").ap()
    wd_d = nc.dram_tensor("wd", [NDG, NFB, P, KO2, P * DG], f32,
                          kind="ExternalInput").ap()
    wr_d = nc.dram_tensor("wrep", [P, cap], f32, kind="ExternalInput").ap()
    out_d = nc.dram_tensor("out_t", [D, cap], f32, kind="ExternalOutput").ap()
    scr_d = nc.dram_tensor("scr", [P, 512], f32).ap()   # warm-up sink

    with tile.TileContext(nc) as tc:
        with (
            tc.tile_pool(name="resident", bufs=1) as rpool,
            tc.tile_pool(name="w1", bufs=3) as w1pool,
            tc.tile_pool(name="wd2", bufs=8) as wd2pool,
            tc.tile_pool(name="outp", bufs=4) as outpool,
            tc.tile_pool(name="psum", bufs=2, space="PSUM") as ppool,
        ):
            xt = rpool.tile([P, KD, cap], bf16)
            wrep = rpool.tile([P, cap], f32)
            nc.sync.dma_start(wrep[:], wr_d)
            at = rpool.tile([P, KF, cap], bf16)

            # Warm-up: the first real matmul can't start until ~17us (DMA
            # latency).  Run throwaway matmuls on a zeroed tile during that
            # window so the PE HAM clock-gate opens (1.2 -> 2.4 GHz) before
            # real work arrives, and the transition timing is deterministic.
            warm = rpool.tile([P, 512], bf16)
            nc.vector.memset(warm[:], 0.0)
            wps = ppool.tile([P, 512], f32, tag="ps0c0", name="warm_ps")
            for i in range(20):
                nc.tensor.matmul(wps[:], warm[:, :P], warm[:],
                                 start=(i == 0), stop=(i == 19))
            wout = rpool.tile([P, 512], f32)
            nc.vector.tensor_copy(out=wout[:], in_=wps[:])
            nc.sync.dma_start(scr_d[:], wout[:])

            # Queue order on the single SWDGE ring decides arrival order:
            # first weight sub-block + first token slices (so PE can start
            # ~13us in), then the token bulk, then the stream.
            nc.gpsimd.dma_start(xt[:, :2, :], xt_d[:, :2, :])

            w1tiles = []
            for fg in range(NFG):
                wgb = w1pool.tile([P, KD, P * FG], bf16, tag="wgb",
                                  name=f"wgb_{fg}")
                wub = w1pool.tile([P, KD, P * FG], bf16, tag="wub",
                                  name=f"wub_{fg}")
                w1tiles.append((wgb, wub))
                if fg == 0:
                    # fine-grained first block + token bulk spread over
                    # several DMAs so multiple SWDGE lanes pull in parallel
                    for s in range(FG):
                        sl = slice(s * P, (s + 1) * P)
                        nc.gpsimd.dma_start(wgb[:, :, sl], wg_d[0][:, :, sl])
                        nc.gpsimd.dma_start(wub[:, :, sl], wu_d[0][:, :, sl])
                        if s == 0:
                            for k0 in range(2, KD, 2):
                                nc.gpsimd.dma_start(
                                    xt[:, k0:k0 + 2, :], xt_d[:, k0:k0 + 2, :])
                else:
                    kh = KD // 2
                    nc.gpsimd.dma_start(wgb[:, :kh, :], wg_d[fg][:, :kh, :])
                    nc.gpsimd.dma_start(wgb[:, kh:, :], wg_d[fg][:, kh:, :])
                    nc.gpsimd.dma_start(wub[:, :kh, :], wu_d[fg][:, :kh, :])
                    nc.gpsimd.dma_start(wub[:, kh:, :], wu_d[fg][:, kh:, :])

                # ---- phase 1: gT/uT = W.T @ xT, aT = silu(gT)*uT ----
                for fs in range(FG):
                    ft = fg * FG + fs
                    for ci, (c0, cn) in enumerate(zip(starts, chunks)):
                        pg = ppool.tile([P, cn], f32, tag=f"ps0c{ci}")
                        pu = ppool.tile([P, cn], f32, tag=f"ps1c{ci}")
                        for ko in range(KD):
                            nc.tensor.matmul(
                                pg[:], wgb[:, ko, fs * P:(fs + 1) * P],
                                xt[:, ko, c0:c0 + cn],
                                start=(ko == 0), stop=(ko == KD - 1))
                        for ko in range(KD):
                            nc.tensor.matmul(
                                pu[:], wub[:, ko, fs * P:(fs + 1) * P],
                                xt[:, ko, c0:c0 + cn],
                                start=(ko == 0), stop=(ko == KD - 1))
                        a_sl = at[:, ft, c0:c0 + cn]
                        nc.scalar.activation(
                            a_sl, pg[:], mybir.ActivationFunctionType.Silu)
                        nc.vector.tensor_tensor(a_sl, a_sl, pu[:], mult)

            # ---- phase 2: dT = Wd.T @ aT, out = dT * w ----
            for dg in range(NDG):
                pds = [[ppool.tile([P, cn], f32, tag=f"ps{ds}c{ci}",
                                   name=f"pd_{dg}_{ds}_{ci}")
                        for ci, cn in enumerate(chunks)]
                       for ds in range(DG)]
                for fb in range(NFB):
                    wdb = wd2pool.tile([P, KO2, P * DG], bf16, tag="wdb")
                    nc.gpsimd.dma_start(wdb[:], wd_d[dg, fb])
                    for ko in range(KO2):
                        fk = fb * KO2 + ko
                        for ds in range(DG):
                            for ci, (c0, cn) in enumerate(zip(starts, chunks)):
                                nc.tensor.matmul(
                                    pds[ds][ci][:],
                                    wdb[:, ko, ds * P:(ds + 1) * P],
                                    at[:, fk, c0:c0 + cn],
                                    start=(fk == 0), stop=(fk == KF - 1))
                for ds in range(DG):
                    ot = outpool.tile([P, cap], f32, tag="ot")
                    for ci, (c0, cn) in enumerate(zip(starts, chunks)):
                        nc.vector.tensor_tensor(
                            ot[:, c0:c0 + cn], pds[ds][ci][:],
                            wrep[:, c0:c0 + cn], mult)
                    dt_idx = dg * DG + ds
                    nc.sync.dma_start(
                        out_d[dt_idx * P:(dt_idx + 1) * P, :], ot[:])

    nc.compile()
    return nc, chunks


def _swizzle_w1(w):
    """[D, F] -> [NFG, P, KD, P*FG] block-major, partition-contiguous."""
    return np.ascontiguousarray(
        w.reshape(KD, P, NFG, P * FG).transpose(2, 1, 0, 3))


def _swizzle_wd(w):
    """[F, D] -> [NDG, NFB, P, KO2, P*DG] block-major."""
    return np.ascontiguousarray(
        w.reshape(NFB, KO2, P, NDG, P * DG).transpose(3, 0, 2, 1, 4))


def kernel(x, gate_tensor, Wg, Wu, Wd):
    global LAST_RESULT
    from concourse.bass_interp import get_hw_module
    from concourse.bass_utils import run_bass_kernel_spmd

    x = np.ascontiguousarray(np.asarray(x, dtype=np.float32))
    gate_tensor = np.asarray(gate_tensor, dtype=np.float32)
    Wg = np.asarray(Wg, dtype=np.float32)
    Wu = np.asarray(Wu, dtype=np.float32)
    Wd = np.asarray(Wd, dtype=np.float32)

    # ---- router (replicated; tiny: T*D*E flops) ----
    logits = x @ gate_tensor                      # [T, E] fp32
    m = logits.max(axis=-1, keepdims=True)
    p = np.exp(logits - m, dtype=np.float32)
    p /= p.sum(axis=-1, keepdims=True)
    topi = np.argsort(-p, axis=-1, kind="stable")[:, :TOPK]      # [T, K]
    topw = np.take_along_axis(p, topi, axis=-1)
    topw = topw / (topw.sum(axis=-1, keepdims=True) + 1e-20)

    idx = []          # tokens routed to each expert
    wts = []          # their combine weights
    for e in range(E):
        sel = (topi == e)                         # [T, K]; <=1 True per row
        idx.append(np.nonzero(sel.any(axis=-1))[0])
        wts.append(topw[sel].astype(np.float32))  # row-major == token order
    max_n = max(len(t) for t in idx)
    cap = max(512, ((max_n + 1) // 2) * 2)

    if cap not in _COMPILED:
        _COMPILED[cap] = _build(cap)
    nc, _chunks = _COMPILED[cap]

    # ---- dispatch: per-core inputs (pre-swizzled to SBUF block layout) ----
    in_maps = []
    for e in range(E):
        n = len(idx[e])
        xg = x[idx[e]]                            # [n, D]
        xt = np.zeros((P, KD, cap), dtype=np.float32)
        xt[:, :, :n] = xg.T.reshape(KD, P, n).transpose(1, 0, 2)
        wr = np.zeros((P, cap), dtype=np.float32)
        wr[:, :n] = wts[e][None, :]
        in_maps.append({"xt": xt, "wg": _swizzle_w1(Wg[e]),
                        "wu": _swizzle_w1(Wu[e]), "wd": _swizzle_wd(Wd[e]),
                        "wrep": wr})

    trace = bool(int(os.environ.get("KERNEL_TRACE", "0")))
    old_m = nc.m
    nc.m = get_hw_module(nc.m)
    try:
        try:
            res = run_bass_kernel_spmd(nc, in_maps, core_ids=list(range(E)),
                                       trace=trace)
        except (ImportError, ModuleNotFoundError):
            # tracing requested (e.g. BASS_TRACE in the env) but this image
            # lacks the axon NTFF profile hook -- rerun without tracing
            os.environ["BASS_NEVER_TRACE"] = "1"
            res = run_bass_kernel_spmd(nc, in_maps, core_ids=list(range(E)),
                                       trace=False)
    finally:
        nc.m = old_m
    LAST_RESULT = res

    # ---- combine: scatter-add the per-expert partials ----
    out = np.zeros((T, D), dtype=np.float32)
    for e in range(E):
        n = len(idx[e])
        out[idx[e]] += res.results[e]["out_t"][:, :n].T
    return out



# revision 2
# speedup vs baseline: 1.0241x; 1.0241x over previous
"""BlockSparseMLP (MoE top-2 routing, 8 experts) — Trainium2 Bass kernel.

Strategy (expert-parallel, per sharding hint): one expert per NeuronCore.
Host-side (numpy): router (x @ gate_tensor, softmax, top-2, renormalize),
token dispatch (gather the tokens routed to each expert, transposed to
feature-major, pre-swizzled into the SBUF block layout and cast to bf16),
and the final combine (combine-weight multiply + scatter-add of the
per-expert partial outputs) — the shard/unshard stage.

Device-side (one SPMD Bass/Tile program on 8 cores): the expert gated MLP
   gT = Wg_e.T @ xT_e   (bf16 matmuls, fp32 PSUM accumulate)
   uT = Wu_e.T @ xT_e
   aT = silu(gT) * uT   (bf16 in SBUF)
   dT = Wd_e.T @ aT     (emitted as bf16)

All inputs are pre-cast to bf16 on the host so HBM traffic is half the
fp32 bytes (weights dominate: 69 MB/core instead of 138 MB), keeping the
SWDGE ring well under saturation while the PE streams at its bf16 rate.
Weights are pre-swizzled on the host into per-DMA-block partition-major
layout so every SWDGE transfer reads large contiguous chunks; the token
axis is the matmul moving dimension, split into chunks <= 512 (PSUM bank
limit).
"""

import os

import ml_dtypes
import numpy as np

T, D, F, E, TOPK = 2048, 2048, 5632, 8, 2
P = 128
KD = D // P     # 16 k-subtiles over D
KF = F // P     # 44 k-subtiles over F
FG = 4          # f-tiles per phase-1 weight DMA block (512 F columns)
NFG = KF // FG  # 11 phase-1 blocks
DG = 2          # d-tiles per phase-2 psum group (256 D columns)
NDG = KD // DG  # 8 phase-2 d-groups
KO2 = 4         # f-subtiles per phase-2 weight DMA block
NFB = KF // KO2  # 11 phase-2 blocks per d-group

BF16 = ml_dtypes.bfloat16

_COMPILED = {}   # CAP -> (nc, chunk list)
LAST_RESULT = None  # BassKernelResults of the most recent run (for test.py)


def _token_chunks(cap):
    """Split cap into free-dim chunks, each in [256, 512]."""
    assert cap >= 512 and cap % 2 == 0
    n512, rem = divmod(cap, 512)
    if rem == 0:
        return [512] * n512
    if rem >= 256:
        return [512] * n512 + [rem]
    return [512] * (n512 - 1) + [256 + rem, 256]


def _build(cap):
    """Build + compile the SPMD Tile program for token capacity `cap`."""
    import concourse.bass as bass  # noqa: F401
    import concourse.mybir as mybir
    import concourse.tile as tile
    from concourse import bacc

    f32 = mybir.dt.float32
    bf16 = mybir.dt.bfloat16
    mult = mybir.AluOpType.mult

    chunks = _token_chunks(cap)
    starts = [sum(chunks[:i]) for i in range(len(chunks))]

    nc = bacc.Bacc("TRN2", target_bir_lowering=False, debug=False,
                   enable_asserts=False, num_devices=E)

    xt_d = nc.dram_tensor("xt", [P, KD, cap], bf16, kind="ExternalInput").ap()
    wg_d = nc.dram_tensor("wg", [NFG, P, KD, P * FG], bf16,
                          kind="ExternalInput").ap()
    wu_d = nc.dram_tensor("wu", [NFG, P, KD, P * FG], bf16,
                          kind="ExternalInput").ap()
    wd_d = nc.dram_tensor("wd", [NDG, NFB, P, KO2, P * DG], bf16,
                          kind="ExternalInput").ap()
    out_d = nc.dram_tensor("out_t", [D, cap], bf16, kind="ExternalOutput").ap()
    scr_d = nc.dram_tensor("scr", [P, 512], f32).ap()   # warm-up sink

    with tile.TileContext(nc) as tc:
        with (
            tc.tile_pool(name="resident", bufs=1) as rpool,
            tc.tile_pool(name="w1", bufs=3) as w1pool,
            tc.tile_pool(name="wd2", bufs=8) as wd2pool,
            tc.tile_pool(name="outp", bufs=4) as outpool,
            tc.tile_pool(name="psum", bufs=2, space="PSUM") as ppool,
        ):
            xt = rpool.tile([P, KD, cap], bf16)
            at = rpool.tile([P, KF, cap], bf16)

            # Warm-up: the first real matmul can't start until ~13us (DMA
            # latency).  Run throwaway matmuls on a zeroed tile during that
            # window so the PE HAM clock-gate opens (1.2 -> 2.4 GHz) before
            # real work arrives, and the transition timing is deterministic.
            warm = rpool.tile([P, 512], bf16)
            nc.vector.memset(warm[:], 0.0)
            wps = ppool.tile([P, 512], f32, tag="ps0c0", name="warm_ps")
            for i in range(20):
                nc.tensor.matmul(wps[:], warm[:, :P], warm[:],
                                 start=(i == 0), stop=(i == 19))
            wout = rpool.tile([P, 512], f32)
            nc.vector.tensor_copy(out=wout[:], in_=wps[:])
            nc.sync.dma_start(scr_d[:], wout[:])

            # Queue order on the single SWDGE ring decides arrival order:
            # first weight sub-block + first token slices (so PE can start
            # early), then the token bulk, then the stream.
            nc.gpsimd.dma_start(xt[:, :2, :], xt_d[:, :2, :])

            w1tiles = []
            for fg in range(NFG):
                wgb = w1pool.tile([P, KD, P * FG], bf16, tag="wgb",
                                  name=f"wgb_{fg}")
                wub = w1pool.tile([P, KD, P * FG], bf16, tag="wub",
                                  name=f"wub_{fg}")
                w1tiles.append((wgb, wub))
                if fg == 0:
                    # fine-grained first block + token bulk spread over
                    # several DMAs so multiple SWDGE lanes pull in parallel
                    for s in range(FG):
                        sl = slice(s * P, (s + 1) * P)
                        nc.gpsimd.dma_start(wgb[:, :, sl], wg_d[0][:, :, sl])
                        nc.gpsimd.dma_start(wub[:, :, sl], wu_d[0][:, :, sl])
                        if s == 0:
                            for k0 in range(2, KD, 2):
                                nc.gpsimd.dma_start(
                                    xt[:, k0:k0 + 2, :], xt_d[:, k0:k0 + 2, :])
                else:
                    kh = KD // 2
                    nc.gpsimd.dma_start(wgb[:, :kh, :], wg_d[fg][:, :kh, :])
                    nc.gpsimd.dma_start(wgb[:, kh:, :], wg_d[fg][:, kh:, :])
                    nc.gpsimd.dma_start(wub[:, :kh, :], wu_d[fg][:, :kh, :])
                    nc.gpsimd.dma_start(wub[:, kh:, :], wu_d[fg][:, kh:, :])

                # ---- phase 1: gT/uT = W.T @ xT, aT = silu(gT)*uT ----
                for fs in range(FG):
                    ft = fg * FG + fs
                    for ci, (c0, cn) in enumerate(zip(starts, chunks)):
                        pg = ppool.tile([P, cn], f32, tag=f"ps0c{ci}")
                        pu = ppool.tile([P, cn], f32, tag=f"ps1c{ci}")
                        for ko in range(KD):
                            nc.tensor.matmul(
                                pg[:], wgb[:, ko, fs * P:(fs + 1) * P],
                                xt[:, ko, c0:c0 + cn],
                                start=(ko == 0), stop=(ko == KD - 1))
                        for ko in range(KD):
                            nc.tensor.matmul(
                                pu[:], wub[:, ko, fs * P:(fs + 1) * P],
                                xt[:, ko, c0:c0 + cn],
                                start=(ko == 0), stop=(ko == KD - 1))
                        a_sl = at[:, ft, c0:c0 + cn]
                        nc.scalar.activation(
                            a_sl, pg[:], mybir.ActivationFunctionType.Silu)
                        nc.vector.tensor_tensor(a_sl, a_sl, pu[:], mult)

            # ---- phase 2: dT = Wd.T @ aT (combine weights applied on host) ----
            for dg in range(NDG):
                pds = [[ppool.tile([P, cn], f32, tag=f"ps{ds}c{ci}",
                                   name=f"pd_{dg}_{ds}_{ci}")
                        for ci, cn in enumerate(chunks)]
                       for ds in range(DG)]
                for fb in range(NFB):
                    wdb = wd2pool.tile([P, KO2, P * DG], bf16, tag="wdb")
                    nc.gpsimd.dma_start(wdb[:], wd_d[dg, fb])
                    for ko in range(KO2):
                        fk = fb * KO2 + ko
                        for ds in range(DG):
                            for ci, (c0, cn) in enumerate(zip(starts, chunks)):
                                nc.tensor.matmul(
                                    pds[ds][ci][:],
                                    wdb[:, ko, ds * P:(ds + 1) * P],
                                    at[:, fk, c0:c0 + cn],
                                    start=(fk == 0), stop=(fk == KF - 1))
                for ds in range(DG):
                    ot = outpool.tile([P, cap], bf16, tag="ot")
                    for ci, (c0, cn) in enumerate(zip(starts, chunks)):
                        nc.vector.tensor_copy(out=ot[:, c0:c0 + cn],
                                              in_=pds[ds][ci][:])
                    dt_idx = dg * DG + ds
                    nc.sync.dma_start(
                        out_d[dt_idx * P:(dt_idx + 1) * P, :], ot[:])

    nc.compile()
    return nc, chunks


def _swizzle_w1(w):
    """[D, F] -> [NFG, P, KD, P*FG] block-major, partition-contiguous."""
    return np.ascontiguousarray(
        w.reshape(KD, P, NFG, P * FG).transpose(2, 1, 0, 3)).astype(BF16)


def _swizzle_wd(w):
    """[F, D] -> [NDG, NFB, P, KO2, P*DG] block-major."""
    return np.ascontiguousarray(
        w.reshape(NFB, KO2, P, NDG, P * DG).transpose(3, 0, 2, 1, 4)).astype(BF16)


def kernel(x, gate_tensor, Wg, Wu, Wd):
    global LAST_RESULT
    from concourse.bass_interp import get_hw_module
    from concourse.bass_utils import run_bass_kernel_spmd

    x = np.ascontiguousarray(np.asarray(x, dtype=np.float32))
    gate_tensor = np.asarray(gate_tensor, dtype=np.float32)
    Wg = np.asarray(Wg, dtype=np.float32)
    Wu = np.asarray(Wu, dtype=np.float32)
    Wd = np.asarray(Wd, dtype=np.float32)

    # ---- router (replicated; tiny: T*D*E flops) ----
    logits = x @ gate_tensor                      # [T, E] fp32
    m = logits.max(axis=-1, keepdims=True)
    p = np.exp(logits - m, dtype=np.float32)
    p /= p.sum(axis=-1, keepdims=True)
    topi = np.argsort(-p, axis=-1, kind="stable")[:, :TOPK]      # [T, K]
    topw = np.take_along_axis(p, topi, axis=-1)
    topw = topw / (topw.sum(axis=-1, keepdims=True) + 1e-20)

    idx = []          # tokens routed to each expert
    wts = []          # their combine weights
    for e in range(E):
        sel = (topi == e)                         # [T, K]; <=1 True per row
        idx.append(np.nonzero(sel.any(axis=-1))[0])
        wts.append(topw[sel].astype(np.float32))  # row-major == token order
    max_n = max(len(t) for t in idx)
    cap = max(512, ((max_n + 1) // 2) * 2)

    if cap not in _COMPILED:
        _COMPILED[cap] = _build(cap)
    nc, _chunks = _COMPILED[cap]

    # ---- dispatch: per-core inputs (pre-swizzled to SBUF block layout) ----
    in_maps = []
    for e in range(E):
        n = len(idx[e])
        xg = x[idx[e]]                            # [n, D]
        xt = np.zeros((P, KD, cap), dtype=BF16)
        xt[:, :, :n] = xg.T.reshape(KD, P, n).transpose(1, 0, 2).astype(BF16)
        in_maps.append({"xt": xt, "wg": _swizzle_w1(Wg[e]),
                        "wu": _swizzle_w1(Wu[e]), "wd": _swizzle_wd(Wd[e])})

    trace = bool(int(os.environ.get("KERNEL_TRACE", "0")))
    old_m = nc.m
    nc.m = get_hw_module(nc.m)
    try:
        try:
            res = run_bass_kernel_spmd(nc, in_maps, core_ids=list(range(E)),
                                       trace=trace)
        except (ImportError, ModuleNotFoundError):
            # tracing requested (e.g. BASS_TRACE in the env) but this image
            # lacks the axon NTFF profile hook -- rerun without tracing
            os.environ["BASS_NEVER_TRACE"] = "1"
            res = run_bass_kernel_spmd(nc, in_maps, core_ids=list(range(E)),
                                       trace=False)
    finally:
        nc.m = old_m
    LAST_RESULT = res

    # ---- combine: weight + scatter-add the per-expert partials ----
    out = np.zeros((T, D), dtype=np.float32)
    for e in range(E):
        n = len(idx[e])
        d = res.results[e]["out_t"][:, :n].astype(np.float32)
        out[idx[e]] += wts[e][None, :].T * d.T
    return out


# revision 4
# speedup vs baseline: 1.0478x; 1.0232x over previous
"""BlockSparseMLP (MoE top-2 routing, 8 experts) — Trainium2 Bass kernel.

Strategy: pairwise expert-tensor-parallelism for load balance.  The
router (host) yields per-expert token counts n_e with max ~546 but mean
512; pure expert-parallel paces all 8 cores at the heaviest expert.
Instead experts are paired heavy+light (greedy: i-th largest with i-th
smallest) and each pair is served by TWO cores, each holding HALF of the
F dimension of BOTH experts' weights (same weight bytes per core as
expert-parallel).  Both cores process all of the pair's tokens on their
F-half; the down-projection is then a partial sum over F, and the host
adds the two cores' partials during the combine (free — no device
collective).  Per-core token slots: [0, capA) heavy expert, [capA,
capA+capB) light expert, capA = max heavy count, capB = max light
count; capA+capB ~ 1056 vs 2*546 = 1092 worth of slot-work for pure
expert-parallel.

Device program per core (SPMD, shapes uniform):
   f-tiles 0..21  = heavy expert's F-half   (tokens [0, capA))
   f-tiles 22..43 = light expert's F-half   (tokens [capA, capA+capB))
   gT = Wg.T @ xT ; uT = Wu.T @ xT ; aT = silu(gT)*uT   (bf16)
   dT_partial = Wd.T @ aT                                (bf16 out)

All inputs are pre-cast to bf16 on the host (HBM read ~74 MB/core) and
pre-swizzled into per-DMA-block partition-major layout.  PSUM: six
exact-size accumulators (A-chunk0 290, A-chunk1 256, B-chunk 510 for
each of g/u) shared by phase 2 (ds=0 reuses the g tags, ds=1 the u
tags) + 1 warm-up bank.
"""

import os

import ml_dtypes
import numpy as np

T, D, F, E, TOPK = 2048, 2048, 5632, 8, 2
P = 128
KD = D // P      # 16 k-subtiles over D
KF = F // P      # 44 f-tiles total (22 per expert F-half)
KFH = KF // 2    # 22
FG = 4           # f-tiles per phase-1 weight DMA block
NFG = KF // FG   # 11 phase-1 blocks
DG = 2           # d-tiles per phase-2 psum group (256 D columns)
NDG = KD // DG   # 8 phase-2 d-groups
KO2 = 4          # f-subtiles per phase-2 weight DMA block
NFB = KF // KO2  # 11 phase-2 blocks per d-group

BF16 = ml_dtypes.bfloat16

_COMPILED = {}   # (capA, capB) -> nc
LAST_RESULT = None  # BassKernelResults of the most recent run (for test.py)


def _chunks(cap):
    """Split cap into moving-dim chunks: single if <=512, else pieces in
    [256, 512] (>=256 keeps LDWEIGHTS hidden under the matmul)."""
    assert cap % 2 == 0
    if cap <= 512:
        return [cap]
    n512, rem = divmod(cap, 512)
    if rem == 0:
        return [512] * n512
    if rem >= 256:
        return [512] * n512 + [rem]
    return [512] * (n512 - 1) + [256 + rem, 256]


def _build(capA, capB):
    """Build + compile the SPMD Tile program."""
    import concourse.bass as bass  # noqa: F401
    import concourse.mybir as mybir
    import concourse.tile as tile
    from concourse import bacc

    f32 = mybir.dt.float32
    bf16 = mybir.dt.bfloat16
    mult = mybir.AluOpType.mult

    cap = capA + capB
    # region r: (first f-tile, first token slot, chunk widths)
    regs = [(0, 0, _chunks(capA)), (KFH, capA, _chunks(capB))]

    def reg_of(ft):
        return regs[0] if ft < KFH else regs[1]

    nc = bacc.Bacc("TRN2", target_bir_lowering=False, debug=False,
                   enable_asserts=False, num_devices=E)

    xt_d = nc.dram_tensor("xt", [P, KD, cap], bf16, kind="ExternalInput").ap()
    wg_d = nc.dram_tensor("wg", [NFG, P, KD, P * FG], bf16,
                          kind="ExternalInput").ap()
    wu_d = nc.dram_tensor("wu", [NFG, P, KD, P * FG], bf16,
                          kind="ExternalInput").ap()
    wd_d = nc.dram_tensor("wd", [NDG, NFB, P, KO2, P * DG], bf16,
                          kind="ExternalInput").ap()
    out_d = nc.dram_tensor("out_t", [D, cap], bf16, kind="ExternalOutput").ap()
    scr_d = nc.dram_tensor("scr", [P, 512], f32).ap()   # warm-up sink

    with tile.TileContext(nc) as tc:
        with (
            tc.tile_pool(name="resident", bufs=1) as rpool,
            tc.tile_pool(name="w1", bufs=3) as w1pool,
            tc.tile_pool(name="wd2", bufs=4) as wd2pool,
            tc.tile_pool(name="outp", bufs=4) as outpool,
            tc.tile_pool(name="psum", bufs=1, space="PSUM") as ppool,
        ):
            xt = rpool.tile([P, KD, cap], bf16)
            # a: A f-tile i in cols [0, capA) of plane i, B f-tile i in
            # cols [capA, cap) of plane i — packed, 22 planes not 44.
            at = rpool.tile([P, KFH, cap], bf16)

            def psum(kind, reg_idx, ci, cn, name=None):
                return ppool.tile([P, cn], f32,
                                  tag=f"{kind}{'AB'[reg_idx]}{ci}", name=name)

            # Warm-up: PE HAM clock ramp (1.2 -> 2.4 GHz) while first DMAs
            # are in flight; sized to end right when the first weight +
            # token DMAs complete (~16.4us: ~9us NEFF preamble + ~7us cold
            # SWDGE latency) so the clock never dips back to half rate.
            NWARM = 23
            warm = rpool.tile([P, 512], bf16)
            nc.vector.memset(warm[:], 0.0)
            wps = ppool.tile([P, 512], f32, tag="warm", name="warm_ps")
            for i in range(NWARM):
                nc.tensor.matmul(wps[:], warm[:, :P], warm[:],
                                 start=(i == 0), stop=(i == NWARM - 1))
            wout = rpool.tile([P, 512], f32)
            nc.vector.tensor_copy(out=wout[:], in_=wps[:])
            nc.sync.dma_start(scr_d[:], wout[:])

            for fg in range(NFG):
                wgb = w1pool.tile([P, KD, P * FG], bf16, tag="wgb",
                                  name=f"wgb_{fg}")
                wub = w1pool.tile([P, KD, P * FG], bf16, tag="wub",
                                  name=f"wub_{fg}")
                if fg == 0:
                    # Queue order on the SWDGE ring decides arrival order.
                    # The first f-tile's 16 k-passes consume ALL of xt, so
                    # xt goes right after the first 128-col wg slice —
                    # otherwise the PE starves ~1us per 2-ktile piece.
                    nc.gpsimd.dma_start(wgb[:, :, :P], wg_d[0][:, :, :P])
                    for k0 in range(0, KD, 2):
                        nc.gpsimd.dma_start(
                            xt[:, k0:k0 + 2, :], xt_d[:, k0:k0 + 2, :])
                    nc.gpsimd.dma_start(wub[:, :, :P], wu_d[0][:, :, :P])
                    for s in range(1, FG):
                        sl = slice(s * P, (s + 1) * P)
                        nc.gpsimd.dma_start(wgb[:, :, sl], wg_d[0][:, :, sl])
                        nc.gpsimd.dma_start(wub[:, :, sl], wu_d[0][:, :, sl])
                else:
                    kh = KD // 2
                    nc.gpsimd.dma_start(wgb[:, :kh, :], wg_d[fg][:, :kh, :])
                    nc.gpsimd.dma_start(wgb[:, kh:, :], wg_d[fg][:, kh:, :])
                    nc.gpsimd.dma_start(wub[:, :kh, :], wu_d[fg][:, :kh, :])
                    nc.gpsimd.dma_start(wub[:, kh:, :], wu_d[fg][:, kh:, :])

                # ---- phase 1: gT/uT = W.T @ xT, aT = silu(gT)*uT ----
                for fs in range(FG):
                    ft = fg * FG + fs
                    f0, t0, cws = reg_of(ft)
                    ridx = 0 if ft < KFH else 1
                    c0 = t0
                    for ci, cn in enumerate(cws):
                        pg = psum("g", ridx, ci, cn, name=f"pg_{ft}_{ci}")
                        pu = psum("u", ridx, ci, cn, name=f"pu_{ft}_{ci}")
                        for ko in range(KD):
                            nc.tensor.matmul(
                                pg[:], wgb[:, ko, fs * P:(fs + 1) * P],
                                xt[:, ko, c0:c0 + cn],
                                start=(ko == 0), stop=(ko == KD - 1))
                        for ko in range(KD):
                            nc.tensor.matmul(
                                pu[:], wub[:, ko, fs * P:(fs + 1) * P],
                                xt[:, ko, c0:c0 + cn],
                                start=(ko == 0), stop=(ko == KD - 1))
                        a_sl = at[:, ft - f0, c0:c0 + cn]
                        nc.scalar.activation(
                            a_sl, pg[:], mybir.ActivationFunctionType.Silu)
                        nc.vector.tensor_tensor(a_sl, a_sl, pu[:], mult)
                        c0 += cn

            # ---- phase 2: dT_partial = Wd.T @ aT (combine on host) ----
            for dg in range(NDG):
                # accumulators: [ds][region][chunk]; ds=0 reuses g tags,
                # ds=1 the u tags (exact same widths)
                pds = [[[psum("gu"[ds], ridx, ci, cn,
                              name=f"pd_{dg}_{ds}_{ridx}_{ci}")
                         for ci, cn in enumerate(regs[ridx][2])]
                        for ridx in range(2)]
                       for ds in range(DG)]
                for fb in range(NFB):
                    wdb = wd2pool.tile([P, KO2, P * DG], bf16, tag="wdb")
                    nc.gpsimd.dma_start(wdb[:], wd_d[dg, fb])
                    for ko in range(KO2):
                        fk = fb * KO2 + ko
                        f0, t0, cws = reg_of(fk)
                        ridx = 0 if fk < KFH else 1
                        for ds in range(DG):
                            c0 = t0
                            for ci, cn in enumerate(cws):
                                nc.tensor.matmul(
                                    pds[ds][ridx][ci][:],
                                    wdb[:, ko, ds * P:(ds + 1) * P],
                                    at[:, fk - f0, c0:c0 + cn],
                                    start=(fk - f0 == 0),
                                    stop=(fk - f0 == KFH - 1))
                                c0 += cn
                for ds in range(DG):
                    ot = outpool.tile([P, cap], bf16, tag="ot")
                    dt_idx = dg * DG + ds
                    orow = out_d[dt_idx * P:(dt_idx + 1) * P, :]
                    # emit the A part as soon as its accumulation is done;
                    # the B part follows (shorter tail on the last group)
                    for ridx in range(2):
                        f0, t0, cws = regs[ridx]
                        c0 = t0
                        for ci, cn in enumerate(cws):
                            nc.vector.tensor_copy(out=ot[:, c0:c0 + cn],
                                                  in_=pds[ds][ridx][ci][:])
                            c0 += cn
                        nc.sync.dma_start(orow[:, t0:c0], ot[:, t0:c0])

    nc.compile()
    return nc


def _swizzle_w1(w):
    """[D, F] -> [NFG, P, KD, P*FG] block-major, partition-contiguous."""
    return np.ascontiguousarray(
        w.reshape(KD, P, NFG, P * FG).transpose(2, 1, 0, 3)).astype(BF16)


def _swizzle_wd(w):
    """[F, D] -> [NDG, NFB, P, KO2, P*DG] block-major."""
    return np.ascontiguousarray(
        w.reshape(NFB, KO2, P, NDG, P * DG).transpose(3, 0, 2, 1, 4)).astype(BF16)


def kernel(x, gate_tensor, Wg, Wu, Wd):
    global LAST_RESULT
    from concourse.bass_interp import get_hw_module
    from concourse.bass_utils import run_bass_kernel_spmd

    x = np.ascontiguousarray(np.asarray(x, dtype=np.float32))
    gate_tensor = np.asarray(gate_tensor, dtype=np.float32)
    Wg = np.asarray(Wg, dtype=np.float32)
    Wu = np.asarray(Wu, dtype=np.float32)
    Wd = np.asarray(Wd, dtype=np.float32)

    # ---- router (replicated; tiny: T*D*E flops) ----
    logits = x @ gate_tensor                      # [T, E] fp32
    m = logits.max(axis=-1, keepdims=True)
    p = np.exp(logits - m, dtype=np.float32)
    p /= p.sum(axis=-1, keepdims=True)
    topi = np.argsort(-p, axis=-1, kind="stable")[:, :TOPK]      # [T, K]
    topw = np.take_along_axis(p, topi, axis=-1)
    topw = topw / (topw.sum(axis=-1, keepdims=True) + 1e-20)

    idx = []          # tokens routed to each expert
    wts = []          # their combine weights
    for e in range(E):
        sel = (topi == e)                         # [T, K]; <=1 True per row
        idx.append(np.nonzero(sel.any(axis=-1))[0])
        wts.append(topw[sel].astype(np.float32))  # row-major == token order

    # ---- pairing: i-th heaviest with i-th lightest ----
    counts = np.array([len(t) for t in idx])
    order = np.argsort(-counts, kind="stable")
    pairs = [(int(order[i]), int(order[E - 1 - i])) for i in range(E // 2)]
    capA = max(2, (int(counts[order[:E // 2]].max()) + 1) // 2 * 2)
    capB = max(2, (int(counts[order[E // 2:]].max()) + 1) // 2 * 2)
    cap = capA + capB

    key = (capA, capB)
    if key not in _COMPILED:
        _COMPILED[key] = _build(capA, capB)
    nc = _COMPILED[key]

    # ---- dispatch: per-core inputs (pre-swizzled, bf16) ----
    halfF = F // 2
    in_maps = []
    for p_i, (eh, el) in enumerate(pairs):
        nh, nl = len(idx[eh]), len(idx[el])
        xt = np.zeros((P, KD, cap), dtype=BF16)
        xt[:, :, :nh] = (x[idx[eh]].T.reshape(KD, P, nh)
                         .transpose(1, 0, 2).astype(BF16))
        xt[:, :, capA:capA + nl] = (x[idx[el]].T.reshape(KD, P, nl)
                                    .transpose(1, 0, 2).astype(BF16))
        for h in range(2):
            hsl = slice(h * halfF, (h + 1) * halfF)
            wg = _swizzle_w1(np.concatenate(
                [Wg[eh][:, hsl], Wg[el][:, hsl]], axis=1))
            wu = _swizzle_w1(np.concatenate(
                [Wu[eh][:, hsl], Wu[el][:, hsl]], axis=1))
            wd = _swizzle_wd(np.concatenate(
                [Wd[eh][hsl, :], Wd[el][hsl, :]], axis=0))
            in_maps.append({"xt": xt, "wg": wg, "wu": wu, "wd": wd})

    trace = bool(int(os.environ.get("KERNEL_TRACE", "0")))
    old_m = nc.m
    nc.m = get_hw_module(nc.m)
    try:
        try:
            res = run_bass_kernel_spmd(nc, in_maps, core_ids=list(range(E)),
                                       trace=trace)
        except (ImportError, ModuleNotFoundError):
            # tracing requested but this image lacks the axon NTFF profile
            # hook -- rerun without tracing
            os.environ["BASS_NEVER_TRACE"] = "1"
            res = run_bass_kernel_spmd(nc, in_maps, core_ids=list(range(E)),
                                       trace=False)
    finally:
        nc.m = old_m
    LAST_RESULT = res

    # ---- combine: add F-half partials, weight, scatter-add ----
    out = np.zeros((T, D), dtype=np.float32)
    for p_i, (eh, el) in enumerate(pairs):
        nh, nl = len(idx[eh]), len(idx[el])
        d = (res.results[2 * p_i]["out_t"].astype(np.float32)
             + res.results[2 * p_i + 1]["out_t"].astype(np.float32))
        out[idx[eh]] += wts[eh][:, None] * d[:, :nh].T
        out[idx[el]] += wts[el][:, None] * d[:, capA:capA + nl].T
    return out
